# revision 10
# baseline (speedup 1.0000x reference)
"""DiscRNNG forward pass on 8 Trainium2 NeuronCores (Bass/Tile).

Strategy (batch=1, strictly sequential recurrence):
  - Three independent single-layer LSTM chains (stack, buffer, history) are
    model-parallel: one chain per NeuronCore (cores 0-2; cores 3-7 run
    redundant replicas so the SPMD program is uniform).
  - Per core: embedding projections + x@wih^T contributions for all T steps
    are precomputed as dense matmuls, then the T=4096 sequential steps run
    with only the h@whh^T matvec + LSTM pointwise ops on the critical path.
  - whh is quantized to fp8-e3m4 (x S, a power of two) so LDWEIGHTS runs at
    4 bytes/lane/cycle via FWL; the 1/S unscale is folded into the gate
    activations' `scale` operand. Gates are host-permuted to (f,i,g,o) so
    the post-matmul pointwise tail is 2 ops (sigmoid(o), mul) - everything
    else overlaps earlier gate-groups' matmuls.
  - Gate psum preloads (x-contribution) are done 32 steps at a time with a
    single GpSimd copy into a full PSUM bank; PE matmuls accumulate onto it
    (start=False).
  - The per-step h history is DMA'd out per 32-step block in AllToAll
    layout; a single in-kernel AllToAll distributes each chain's history
    shards so every core computes the softmax head for its own T/8 shard.
    One SPMD launch total.
Embedding gather (4096 rows of the 100k x 300 table) is done host-side.
"""

import os
import sys

sys.path.insert(0, "/opt/trn_rl_repo")

import numpy as np
import ml_dtypes

import concourse.bass as bass
import concourse.mybir as mybir
import concourse.tile as tile
import bass_rust

F16 = mybir.dt.float16
F32 = mybir.dt.float32
AF = mybir.ActivationFunctionType

# whh dtype selector: f8e3 (e3m4), f8e4 (e4m3), f16
WHH_DT = os.environ.get("WHH_DT", "f8e3")
_WHH_TABLE = {
    "f8e3": (mybir.dt.float8e3, ml_dtypes.float8_e3m4, 14.0, 15.5),
    "f8e4": (mybir.dt.float8e4, ml_dtypes.float8_e4m3, 200.0, 240.0),
    "f16": (mybir.dt.float16, np.float16, None, None),
}
F8, _WHH_NP, _WHH_TARGET, _WHH_CLIP = _WHH_TABLE[WHH_DT]

T, H, G, E, X2D, NA = 4096, 512, 2048, 512, 1024, 100
U = 32
KC = H // 128        # 4
MC = G // 128        # 16
EC = E // 128        # 4
XC2 = X2D // 128     # 8
TCH = 512            # dense precompute time chunk
NCORES = 8
TS = T // NCORES     # 512, head time shard per core
KC2 = 3 * H // 128   # 12, head contraction tiles
DC = H // 128        # 4
TC = TS // 128       # 4


def _split_excess_waits(nc, maxw=1):
    """walrus here allows only 1 sync-wait per instruction; hoist excess
    waits onto preceding same-engine nops."""
    for bb in nc.m.functions[0].blocks:
        insts = list(bb.instructions)
        out = []
        changed = False
        for inst in insts:
            si = inst.sync_info
            if si is not None and si.on_wait is not None and len(si.on_wait) > maxw:
                waits = list(si.on_wait)
                keep = waits[-maxw:]
                excess = waits[:-maxw]
                for i in range(0, len(excess), maxw):
                    chunk = excess[i : i + maxw]
                    nop = nc.engines[inst.engine].nop(hint="waitsplit", nofuse=True).ins
                    cur = nc.cur_bb.bb
                    lst = list(cur.instructions)
                    assert lst and lst[-1].name == nop.name
                    cur.instructions = lst[:-1]
                    nop.sync_info = bass_rust.SyncInfo(
                        on_wait=list(chunk), on_update=[]
                    )
                    out.append(nop)
                si.on_wait = keep
                inst.sync_info = si
                changed = True
            out.append(inst)
        if changed:
            bb.instructions = out


def _build_fused():
    nc = bass.Bass("TRN2", target_bir_lowering=False, debug=False, num_devices=NCORES)

    ecatT = nc.dram_tensor("ecatT", [E, T], F16, kind="ExternalInput").ap()
    wprojT = nc.dram_tensor("wprojT", [E, X2D], F16, kind="ExternalInput").ap()
    bproj = nc.dram_tensor("bproj", [X2D, 1], F32, kind="ExternalInput").ap()
    wih2T = nc.dram_tensor("wih2T", [X2D, G], F16, kind="ExternalInput").ap()
    bias2 = nc.dram_tensor("bias2", [G, 1], F32, kind="ExternalInput").ap()
    whhT8 = nc.dram_tensor("whhT8", [H, G], F8, kind="ExternalInput").ap()
    invs = nc.dram_tensor("invs", [128, 1], F32, kind="ExternalInput").ap()
    h0 = nc.dram_tensor("h0", [128, KC], F32, kind="ExternalInput").ap()
    c0 = nc.dram_tensor("c0", [128, KC], F32, kind="ExternalInput").ap()
    sum_wT = nc.dram_tensor("sum_wT", [3 * H, H], F16, kind="ExternalInput").ap()
    sum_b = nc.dram_tensor("sum_b", [H, 1], F32, kind="ExternalInput").ap()
    out_wT = nc.dram_tensor("out_wT", [H, NA], F16, kind="ExternalInput").ap()
    out_bt = nc.dram_tensor("out_bt", [128, NA], F32, kind="ExternalInput").ap()

    xct_d = nc.dram_tensor("xct", [MC, 128, T + 2 * U], F32).ap()
    hist_d = nc.dram_tensor("hist", [KC, 128, T], F16).ap()
    outd = nc.dram_tensor("logp", [TS, NA], F32, kind="ExternalOutput").ap()

    with tile.TileContext(nc) as tc:
        with (
            tc.tile_pool(name="wts", bufs=1) as wts,
            tc.tile_pool(name="x2p", bufs=2) as x2p,
            tc.tile_pool(name="ps", bufs=2, space="PSUM") as psp,
            tc.tile_pool(name="state", bufs=1) as statep,
            tc.tile_pool(name="xcb", bufs=1) as xcbp,
            tc.tile_pool(name="histb", bufs=1) as histbp,
            tc.tile_pool(name="gps", bufs=1, space="PSUM") as gpsp,
            tc.tile_pool(name="p2", bufs=2, space="PSUM") as p2p,
            tc.tile_pool(name="ew", bufs=4) as ewp,
            tc.tile_pool(name="dram", bufs=1, space="DRAM") as dramp,
        ):
            ecat_sb = wts.tile([128, EC * T], F16)
            nc.sync.dma_start(
                ecat_sb[:].rearrange("p (kx t) -> p kx t", kx=EC),
                ecatT.rearrange("(kx p) t -> p kx t", p=128),
            )
            wproj_sb = wts.tile([128, EC * X2D], F16)
            nc.sync.dma_start(
                wproj_sb[:].rearrange("p (kx m) -> p kx m", kx=EC),
                wprojT.rearrange("(kx p) m -> p kx m", p=128),
            )
            bproj_sb = wts.tile([128, XC2], F32)
            nc.sync.dma_start(
                bproj_sb[:].rearrange("p (c o) -> p c o", o=1),
                bproj.rearrange("(c p) o -> p c o", p=128),
            )
            wih2_sb = wts.tile([128, XC2 * G], F16)
            nc.sync.dma_start(
                wih2_sb[:].rearrange("p (kx m) -> p kx m", kx=XC2),
                wih2T.rearrange("(kx p) m -> p kx m", p=128),
            )
            bias2_sb = wts.tile([128, MC], F32)
            nc.sync.dma_start(
                bias2_sb[:].rearrange("p (c o) -> p c o", o=1),
                bias2.rearrange("(c p) o -> p c o", p=128),
            )
            whh8_sb = wts.tile([128, KC * G], F8)
            nc.sync.dma_start(
                whh8_sb[:].rearrange("p (kc m) -> p kc m", kc=KC),
                whhT8.rearrange("(kc p) m -> p kc m", p=128),
            )
            invs_sb = wts.tile([128, 1], F32)
            nc.sync.dma_start(invs_sb[:], invs)
            # head weights
            sw_sb = wts.tile([128, KC2 * H], F16)
            nc.sync.dma_start(
                sw_sb[:].rearrange("p (k m) -> p k m", k=KC2),
                sum_wT.rearrange("(k p) m -> p k m", p=128),
            )
            sb_sb = wts.tile([128, DC], F32)
            nc.sync.dma_start(
                sb_sb[:].rearrange("p (c o) -> p c o", o=1),
                sum_b.rearrange("(c p) o -> p c o", p=128),
            )
            ow_sb = wts.tile([128, DC * NA], F16)
            nc.sync.dma_start(
                ow_sb[:].rearrange("p (c a) -> p c a", c=DC),
                out_wT.rearrange("(c p) a -> p c a", p=128),
            )
            ob_sb = wts.tile([128, NA], F32)
            nc.sync.dma_start(ob_sb[:], out_bt)

            # precompute XCT = WIH2 @ relu(Wproj @ ecatT + bproj) + bias2
            for tc_i in range(T // TCH):
                tsl = slice(tc_i * TCH, (tc_i + 1) * TCH)
                x2_sb = x2p.tile([128, XC2 * TCH], F16)
                for mx in range(XC2):
                    ps = psp.tile([128, TCH], F32)
                    for kx in range(EC):
                        nc.tensor.matmul(
                            ps[:],
                            wproj_sb[
                                :, kx * X2D + mx * 128 : kx * X2D + (mx + 1) * 128
                            ],
                            ecat_sb[:, kx * T + tc_i * TCH : kx * T + (tc_i + 1) * TCH],
                            start=(kx == 0),
                            stop=(kx == EC - 1),
                        )
                    nc.scalar.activation(
                        x2_sb[:, mx * TCH : (mx + 1) * TCH],
                        ps[:],
                        AF.Relu,
                        bias=bproj_sb[:, mx : mx + 1],
                    )
                for m in range(MC):
                    ps = psp.tile([128, TCH], F32)
                    for kx in range(XC2):
                        nc.tensor.matmul(
                            ps[:],
                            wih2_sb[:, kx * G + m * 128 : kx * G + (m + 1) * 128],
                            x2_sb[:, kx * TCH : (kx + 1) * TCH],
                            start=(kx == 0),
                            stop=(kx == XC2 - 1),
                        )
                    xct_t = x2p.tile([128, TCH], F32, tag="xctout")
                    nc.scalar.activation(
                        xct_t[:], ps[:], AF.Identity, bias=bias2_sb[:, m : m + 1]
                    )
                    nc.sync.dma_start(xct_d[m, :, tsl], xct_t[:])

            # sequential recurrence, software-pipelined XC prefetch
            h_cur = statep.tile([128, KC], F16)
            c_sb = statep.tile([128, KC], F32)
            tmp32 = statep.tile([128, KC], F32)
            nc.sync.dma_start(tmp32[:], h0[:])
            nc.vector.tensor_copy(h_cur[:], tmp32[:])
            nc.sync.dma_start(c_sb[:], c0[:])

            xcA = xcbp.tile([128, MC * U], F32, tag="xcA")
            xcB = xcbp.tile([128, MC * U], F32, tag="xcB")
            nc.sync.dma_start(
                xcA[:].rearrange("p (m u) -> p m u", m=MC),
                xct_d[:, :, 0:U].rearrange("m p u -> p m u"),
            )

            invs_ap = invs_sb[:, 0:1]

            # persistent gate-psum banks (one per half); preloaded with the
            # 32 steps' x-contributions (gate-interleaved layout: col u*16+m),
            # PE accumulates on top with start=False. The preload for each
            # bank is emitted mid-way through the *other* half's step loop.
            gpsA = gpsp.tile([128, U * 16], F32, tag="psA")
            gpsB = gpsp.tile([128, U * 16], F32, tag="psB")

            def preload(psb, xc_sb):
                nc.vector.tensor_copy(
                    psb[:].rearrange("p (u m) -> p u m", m=16),
                    xc_sb[:].rearrange("p (m u) -> p u m", m=MC),
                )

            preload(gpsA, xcA)

            def half(xc_sb, psb, hist_ap, tag, pre_next):
                hist_t = histbp.tile([128, KC * U], F16, tag="h" + tag)
                hist_r = hist_t[:].rearrange("p (k u) -> p u k", k=KC)
                nc.vector.tensor_copy(hist_r[:, 0, :], h_cur[:])
                for u in range(U):
                    if u == 8:
                        pre_next()
                    base = u * 16
                    for grp in range(4):  # f, i, g, o
                        for m in range(grp * 4, grp * 4 + 4):
                            for kc in range(KC):
                                nc.tensor.matmul(
                                    psb[:, base + m : base + m + 1],
                                    whh8_sb[
                                        :, kc * G + m * 128 : kc * G + (m + 1) * 128
                                    ],
                                    hist_t[:, kc * U + u : kc * U + u + 1],
                                    start=False,
                                    stop=(kc == KC - 1),
                                )
                        if grp == 0:
                            sf = ewp.tile([128, 4], F32, tag="sf")
                            nc.scalar.activation(
                                sf[:], psb[:, base : base + 4], AF.Sigmoid,
                                scale=invs_ap,
                            )
                            t2 = ewp.tile([128, 4], F32, tag="t2")
                            nc.vector.tensor_mul(t2[:], sf[:], c_sb[:])
                        elif grp == 1:
                            si = ewp.tile([128, 4], F32, tag="si")
                            nc.scalar.activation(
                                si[:], psb[:, base + 4 : base + 8], AF.Sigmoid,
                                scale=invs_ap,
                            )
                        elif grp == 2:
                            tg = ewp.tile([128, 4], F32, tag="tg")
                            nc.scalar.activation(
                                tg[:], psb[:, base + 8 : base + 12], AF.Tanh,
                                scale=invs_ap,
                            )
                            t1 = ewp.tile([128, 4], F32, tag="t1")
                            nc.vector.tensor_mul(t1[:], si[:], tg[:])
                            nc.vector.tensor_add(c_sb[:], t1[:], t2[:])
                            tc2 = ewp.tile([128, 4], F32, tag="tc2")
                            nc.scalar.activation(tc2[:], c_sb[:], AF.Tanh)
                        else:
                            so = ewp.tile([128, 4], F32, tag="so")
                            nc.scalar.activation(
                                so[:], psb[:, base + 12 : base + 16], AF.Sigmoid,
                                scale=invs_ap,
                            )
                            if u < U - 1:
                                nc.vector.tensor_mul(
                                    hist_r[:, u + 1, :], so[:], tc2[:]
                                )
                            else:
                                nc.vector.tensor_mul(h_cur[:], so[:], tc2[:])
                nc.sync.dma_start(
                    hist_ap.rearrange("k p u -> p k u"),
                    hist_t[:].rearrange("p (k u) -> p k u", k=KC),
                )

            with tc.For_i(0, T, 2 * U, hint_engines=(mybir.EngineType.PE,)) as iv:
                nc.sync.dma_start(
                    xcB[:].rearrange("p (m u) -> p m u", m=MC),
                    xct_d[:, :, U:][:, :, bass.ds(iv, U)].rearrange("m p u -> p m u"),
                )
                half(
                    xcA, gpsA, hist_d[:, :, bass.ds(iv, U)], "A",
                    lambda: preload(gpsB, xcB),
                )
                nc.sync.dma_start(
                    xcA[:].rearrange("p (m u) -> p m u", m=MC),
                    xct_d[:, :, 2 * U :][:, :, bass.ds(iv, U)].rearrange(
                        "m p u -> p m u"
                    ),
                )
                half(
                    xcB, gpsB, hist_d[:, :, U:][:, :, bass.ds(iv, U)], "B",
                    lambda: preload(gpsA, xcA),
                )

            # re-layout history into AllToAll shard-major order and exchange:
            # after A2A, out[r] on core c holds rank r's history columns for
            # time shard c. Ranks 0-2 are chains stk/buf/hist.
            a2a_in = dramp.tile([NCORES, KC, 128, TS], F16)
            a2a_out = dramp.tile([NCORES, KC, 128, TS], F16)
            for s in range(NCORES):
                nc.sync.dma_start(a2a_in[s], hist_d[:, :, s * TS : (s + 1) * TS])
            nc.gpsimd.collective_compute(
                "AllToAll",
                mybir.AluOpType.bypass,
                replica_groups=[list(range(NCORES))],
                ins=[a2a_in.opt()],
                outs=[a2a_out.opt()],
            )

            # softmax head on this core's T/8 shard
            top_sb = wts.tile([128, KC2 * TS], F16)
            top_r = top_sb[:].rearrange("p (k t) -> p k t", k=KC2)
            for q in range(3):
                nc.sync.dma_start(
                    top_r[:, q * KC : (q + 1) * KC, :],
                    a2a_out[q].rearrange("k p t -> p k t"),
                )
            st_sb = wts.tile([128, DC * TS], F16)
            for dc in range(DC):
                ps = psp.tile([128, TS], F32)
                for kc in range(KC2):
                    nc.tensor.matmul(
                        ps[:],
                        sw_sb[:, kc * H + dc * 128 : kc * H + (dc + 1) * 128],
                        top_sb[:, kc * TS : (kc + 1) * TS],
                        start=(kc == 0),
                        stop=(kc == KC2 - 1),
                    )
                nc.scalar.activation(
                    st_sb[:, dc * TS : (dc + 1) * TS],
                    ps[:],
                    AF.Tanh,
                    bias=sb_sb[:, dc : dc + 1],
                )
            for tcc in range(TC):
                ps2 = p2p.tile([128, NA], F32)
                for dc in range(DC):
                    nc.tensor.matmul(
                        ps2[:],
                        st_sb[:, dc * TS + tcc * 128 : dc * TS + tcc * 128 + 128],
                        ow_sb[:, dc * NA : (dc + 1) * NA],
                        start=(dc == 0),
                        stop=(dc == DC - 1),
                    )
                L = ewp.tile([128, NA], F32, tag="hL")
                nc.vector.tensor_add(L[:], ps2[:], ob_sb[:])
                mx = ewp.tile([128, 1], F32, tag="hmx")
                nc.vector.reduce_max(mx[:], L[:], axis=mybir.AxisListType.X)
                D = ewp.tile([128, NA], F32, tag="hD")
                nc.vector.tensor_scalar(
                    D[:], L[:], mx[:], None, mybir.AluOpType.subtract
                )
                Ex = ewp.tile([128, NA], F32, tag="hE")
                nc.scalar.activation(Ex[:], D[:], AF.Exp)
                s = ewp.tile([128, 1], F32, tag="hs")
                nc.vector.reduce_sum(s[:], Ex[:], axis=mybir.AxisListType.X)
                ls = ewp.tile([128, 1], F32, tag="hls")
                nc.scalar.activation(ls[:], s[:], AF.Ln)
                O = ewp.tile([128, NA], F32, tag="hO")
                nc.vector.tensor_scalar(
                    O[:], D[:], ls[:], None, mybir.AluOpType.subtract
                )
                nc.sync.dma_start(outd[tcc * 128 : (tcc + 1) * 128, :], O[:])

    _split_excess_waits(nc)
    return nc


def _make_runner(nc, n_cores=8):
    import jax
    from jax.sharding import Mesh, PartitionSpec
    from jax.experimental.shard_map import shard_map
    from concourse import bass2jax
    from concourse.bass2jax import _bass_exec_p, partition_id_tensor

    bass2jax.install_neuronx_cc_hook()

    partition_name = nc.partition_id_tensor.name if nc.partition_id_tensor else None
    in_names, out_names, out_avals, zero_outs = [], [], [], []
    for alloc in nc.m.functions[0].allocations:
        if not isinstance(alloc, mybir.MemoryLocationSet):
            continue
        name = alloc.memorylocations[0].name
        if alloc.kind == "ExternalInput":
            if name != partition_name:
                in_names.append(name)
        elif alloc.kind == "ExternalOutput":
            shape = tuple(alloc.tensor_shape)
            dtype = mybir.dt.np(alloc.dtype)
            out_names.append(name)
            out_avals.append(jax.core.ShapedArray(shape, dtype))
            zero_outs.append(np.zeros(shape, dtype))
    n_params = len(in_names)
    all_in = list(in_names) + list(out_names) + (
        [partition_name] if partition_name else []
    )

    def _body(*args):
        operands = list(args)
        if partition_name:
            operands.append(partition_id_tensor())
        return tuple(
            _bass_exec_p.bind(
                *operands,
                out_avals=tuple(out_avals),
                in_names=tuple(all_in),
                out_names=tuple(out_names),
                lowering_input_output_aliases=(),
                sim_require_finite=True,
                sim_require_nnan=True,
                nc=nc,
            )
        )

    devices = jax.devices()[:n_cores]
    mesh = Mesh(np.asarray(devices), ("core",))
    nio = n_params + len(out_names)
    fn = jax.jit(
        shard_map(
            _body,
            mesh=mesh,
            in_specs=(PartitionSpec("core"),) * nio,
            out_specs=(PartitionSpec("core"),) * len(out_names),
            check_rep=False,
        ),
        keep_unused=True,
    )

    def run(in_maps):
        import jax

        per_core = [[np.asarray(m[k]) for k in in_names] for m in in_maps]
        concat_in = [
            np.concatenate([per_core[c][i] for c in range(n_cores)], axis=0)
            for i in range(n_params)
        ]
        concat_zeros = [
            np.zeros((n_cores * z.shape[0], *z.shape[1:]), z.dtype)
            for z in zero_outs
        ]
        out = fn(*(concat_in + concat_zeros))
        jax.block_until_ready(out)
        return [
            {
                name: np.asarray(out[i]).reshape(n_cores, *out_avals[i].shape)[c]
                for i, name in enumerate(out_names)
            }
            for c in range(n_cores)
        ]

    run.fn = fn
    run.spec = (in_names, out_names, out_avals, zero_outs, n_cores)
    return run


_CACHE = {}


def _runner():
    if "f" not in _CACHE:
        _CACHE["f"] = _make_runner(_build_fused())
    return _CACHE["f"]


# gate-order permutation (i,f,g,o) -> (f,i,g,o), applied to weight rows
_PERM = np.concatenate(
    [np.arange(512, 1024), np.arange(0, 512), np.arange(1024, 1536),
     np.arange(1536, 2048)]
)


def _prep_cell(inputs, pre, kind, ecat, head):
    wih = np.asarray(inputs[f"{pre}_wih"], np.float32)[_PERM]
    whh = np.asarray(inputs[f"{pre}_whh"], np.float32)[_PERM]
    bias = (
        np.asarray(inputs[f"{pre}_bih"], np.float32)
        + np.asarray(inputs[f"{pre}_bhh"], np.float32)
    )[_PERM]

    # fp8 scale: power of two, scaled |whh*S| kept under the format max
    if _WHH_TARGET is None:
        S = 1.0
        whh8 = (whh).astype(_WHH_NP)
    else:
        wmax = max(float(np.abs(whh).max()), 1e-6)
        S = float(2.0 ** np.floor(np.log2(_WHH_TARGET / wmax)))
        whh8 = np.clip(whh * S, -_WHH_CLIP, _WHH_CLIP).astype(_WHH_NP)

    wih2 = np.zeros((G, X2D), np.float32)
    if kind == "w":
        wih2[:, 0:H] = wih * S
    else:
        wih2[:, H : H + H] = wih * S

    wproj = np.zeros((X2D, E), np.float32)
    wproj[0:512, 0:332] = np.asarray(inputs["w2e_w"])
    wproj[512:1024, 332:396] = np.asarray(inputs["a2e_w"])
    bproj = np.concatenate(
        [np.asarray(inputs["w2e_b"]), np.asarray(inputs["a2e_b"])]
    ).astype(np.float32)

    return {
        "ecatT": np.ascontiguousarray(ecat.T).astype(np.float16),
        "wprojT": np.ascontiguousarray(wproj.T).astype(np.float16),
        "bproj": bproj.reshape(X2D, 1),
        "wih2T": np.ascontiguousarray(wih2.T).astype(np.float16),
        "bias2": (bias * S).astype(np.float32).reshape(G, 1),
        "whhT8": np.ascontiguousarray(whh8.T),
        "invs": np.full((128, 1), 1.0 / S, np.float32),
        "h0": np.ascontiguousarray(
            np.asarray(inputs[f"{pre}_h0"]).reshape(4, 128).T
        ).astype(np.float32),
        "c0": np.ascontiguousarray(
            np.asarray(inputs[f"{pre}_c0"]).reshape(4, 128).T
        ).astype(np.float32),
        **head,
    }


def _prep_inputs(inputs):
    words = np.asarray(inputs["words"]).astype(np.int64)
    pos_tags = np.asarray(inputs["pos_tags"]).astype(np.int64)
    actions = np.asarray(inputs["actions"]).astype(np.int64)

    # host-side embedding gather (4096 of 100k rows), zero-padded to 512
    ecat = np.zeros((T, E), np.float32)
    ecat[:, 0:300] = np.asarray(inputs["word_emb"])[words]
    ecat[:, 300:332] = np.asarray(inputs["pos_emb"])[pos_tags]
    ecat[:, 332:396] = np.asarray(inputs["act_emb"])[actions]

    head = dict(
        sum_wT=np.ascontiguousarray(np.asarray(inputs["sum_w"]).T).astype(
            np.float16
        ),
        sum_b=np.asarray(inputs["sum_b"]).reshape(H, 1).astype(np.float32),
        out_wT=np.ascontiguousarray(np.asarray(inputs["out_w"]).T).astype(
            np.float16
        ),
        out_bt=np.broadcast_to(np.asarray(inputs["out_b"]), (128, NA))
        .astype(np.float32)
        .copy(),
    )
    cells = [("stk", "w"), ("buf", "w"), ("hist", "a")]
    return [
        _prep_cell(inputs, *cells[c % 3], ecat=ecat, head=head)
        for c in range(NCORES)
    ]


def kernel(**inputs):
    run = _runner()
    in_maps = _prep_inputs(inputs)
    res = run(in_maps)
    return np.concatenate([res[c]["logp"] for c in range(NCORES)], axis=0).astype(
        np.float32
    )


# revision 15
# speedup vs baseline: 1.3847x; 1.3847x over previous
"""DiscRNNG forward pass on 8 Trainium2 NeuronCores (Bass/Tile).

Strategy (batch=1, strictly sequential recurrence):
  - Three independent single-layer LSTM chains (stack, buffer, history) are
    model-parallel: one chain per NeuronCore (cores 3-7 run redundant
    replicas so the SPMD program is uniform).
  - Per-launch input upload is the dominant cost in this environment
    (~5 GB/s for per-core inputs, ~8x cheaper for replicated ones), so
    inputs are aggressively compressed: embeddings / weights quantized to
    fp8-e3m4 with runtime power-of-two scales folded into activation
    `scale` operands, and everything identical-across-cores is passed
    replicated (PartitionSpec()) - including the weights of all 3 cells,
    stacked; each core picks its cell with 3 DVE ops driven by a tiny
    per-core one-hot (exact in fp8 since the mask is 0/1).
  - Per core: embedding projections + x@wih^T contributions for all T steps
    are precomputed as dense matmuls into DRAM (device DMA is fast), then
    the T=4096 sequential steps run with only the h@whh^T matvec + LSTM
    pointwise ops on the critical path. whh is fp8 so LDWEIGHTS runs at FWL
    rate; gates are host-permuted to (f,i,g,o) so the post-matmul pointwise
    tail is short. Gate psum x-contribution preloads are done 32 steps at a
    time with a single DVE copy into a PSUM bank; PE matmuls accumulate
    onto it (start=False).
  - The per-step h history (fp16 in SBUF for full recurrence precision) is
    exported per 32-step block as fp8-e4m3 (x128) and exchanged with a
    single in-kernel AllToAll so every core computes the softmax head for
    its own T/8 shard. One SPMD launch total.
Embedding gather (4096 rows of the 100k x 300 table) is done host-side.
"""

import os
import sys

sys.path.insert(0, "/opt/trn_rl_repo")

import numpy as np
import ml_dtypes

import concourse.bass as bass
import concourse.mybir as mybir
import concourse.tile as tile
import bass_rust

F8 = mybir.dt.float8e3
F8E4 = mybir.dt.float8e4
F16 = mybir.dt.float16
F32 = mybir.dt.float32
AF = mybir.ActivationFunctionType
MUL = mybir.AluOpType.mult
ADD = mybir.AluOpType.add
E3M4 = ml_dtypes.float8_e3m4
E4M3 = ml_dtypes.float8_e4m3

T, H, G, E, NA = 4096, 512, 2048, 512, 100
X2E = 512            # effective x2 width per cell (w-cells use w2e rows,
                     # the a-cell uses a2e rows)
U = 32
KC = H // 128        # 4
MC = G // 128        # 16
EC = E // 128        # 4
XC2 = X2E // 128     # 4
TCH = 512            # dense precompute time chunk
NCORES = 8
TS = T // NCORES     # 512, head time shard per core
KC2 = 3 * H // 128   # 12, head contraction tiles
DC = H // 128        # 4
TC = TS // 128       # 4
SH = 128.0           # hist fp8 export scale (|h| < 1 always)


def _split_excess_waits(nc, maxw=1):
    """walrus here allows only 1 sync-wait per instruction; hoist excess
    waits onto preceding same-engine nops."""
    for bb in nc.m.functions[0].blocks:
        insts = list(bb.instructions)
        out = []
        changed = False
        for inst in insts:
            si = inst.sync_info
            if si is not None and si.on_wait is not None and len(si.on_wait) > maxw:
                waits = list(si.on_wait)
                keep = waits[-maxw:]
                excess = waits[:-maxw]
                for i in range(0, len(excess), maxw):
                    chunk = excess[i : i + maxw]
                    nop = nc.engines[inst.engine].nop(hint="waitsplit", nofuse=True).ins
                    cur = nc.cur_bb.bb
                    lst = list(cur.instructions)
                    assert lst and lst[-1].name == nop.name
                    cur.instructions = lst[:-1]
                    nop.sync_info = bass_rust.SyncInfo(
                        on_wait=list(chunk), on_update=[]
                    )
                    out.append(nop)
                si.on_wait = keep
                inst.sync_info = si
                changed = True
            out.append(inst)
        if changed:
            bb.instructions = out


# inputs identical on every core - passed replicated (one upload, not 8)
REPL = {
    "ecatT", "sum_wT", "out_wT", "out_bt", "sum_b", "scl3",
    "whh3", "wih3", "wproj3", "bias2_3", "bproj3",
}


def _build_fused(t_loop=T, pointwise=True):
    nc = bass.Bass("TRN2", target_bir_lowering=False, debug=False, num_devices=NCORES)

    # replicated
    ecatT = nc.dram_tensor("ecatT", [E, T], F8, kind="ExternalInput").ap()
    sum_wT = nc.dram_tensor("sum_wT", [3 * H, H], F8, kind="ExternalInput").ap()
    out_wT = nc.dram_tensor("out_wT", [H, NA], F16, kind="ExternalInput").ap()
    out_bt = nc.dram_tensor("out_bt", [128, NA], F32, kind="ExternalInput").ap()
    sum_b = nc.dram_tensor("sum_b", [H, 1], F32, kind="ExternalInput").ap()
    # per-cell weight stacks, replicated; cols of scl3: invs[3], xsc[3], esc, hsc
    whh3 = nc.dram_tensor("whh3", [3, H, G], F8, kind="ExternalInput").ap()
    wih3 = nc.dram_tensor("wih3", [3, X2E, G], F8, kind="ExternalInput").ap()
    wproj3 = nc.dram_tensor("wproj3", [3, E, X2E], F16, kind="ExternalInput").ap()
    bias2_3 = nc.dram_tensor("bias2_3", [3, G, 1], F32, kind="ExternalInput").ap()
    bproj3 = nc.dram_tensor("bproj3", [3, X2E, 1], F32, kind="ExternalInput").ap()
    scl3 = nc.dram_tensor("scl3", [128, 8], F32, kind="ExternalInput").ap()
    # per-core
    selv = nc.dram_tensor("selv", [128, 3], F32, kind="ExternalInput").ap()
    h0 = nc.dram_tensor("h0", [128, KC], F32, kind="ExternalInput").ap()
    c0 = nc.dram_tensor("c0", [128, KC], F32, kind="ExternalInput").ap()

    xct_d = nc.dram_tensor("xct", [MC, 128, T + 2 * U], F32).ap()
    hist_d = nc.dram_tensor("hist", [KC, 128, T], F8E4).ap()
    outd = nc.dram_tensor("logp", [TS, NA], F32, kind="ExternalOutput").ap()

    with tile.TileContext(nc) as tc:
        with (
            tc.tile_pool(name="wts", bufs=1) as wts,
            tc.tile_pool(name="x2p", bufs=2) as x2p,
            tc.tile_pool(name="ps", bufs=2, space="PSUM") as psp,
            tc.tile_pool(name="state", bufs=1) as statep,
            tc.tile_pool(name="xcb", bufs=1) as xcbp,
            tc.tile_pool(name="histb", bufs=1) as histbp,
            tc.tile_pool(name="gps", bufs=1, space="PSUM") as gpsp,
            tc.tile_pool(name="p2", bufs=2, space="PSUM") as p2p,
            tc.tile_pool(name="ew", bufs=4) as ewp,
            tc.tile_pool(name="dram", bufs=1, space="DRAM") as dramp,
        ):
            ecat_sb = wts.tile([128, EC * T], F8)
            nc.sync.dma_start(
                ecat_sb[:].rearrange("p (kx t) -> p kx t", kx=EC),
                ecatT.rearrange("(kx p) t -> p kx t", p=128),
            )
            # selected cell weights (targets of the one-hot combine)
            wproj_sb = wts.tile([128, EC * X2E], F16)
            bproj_sb = wts.tile([128, XC2], F32)
            wih2_sb = wts.tile([128, XC2 * G], F8)
            bias2_sb = wts.tile([128, MC], F32)
            whh8_sb = wts.tile([128, KC * G], F8)
            sc_sb = wts.tile([128, 4], F32)  # invs, xsc, esc, hsc
            selv_sb = wts.tile([128, 3], F32)
            nc.sync.dma_start(selv_sb[:], selv)
            scl3_sb = wts.tile([128, 8], F32)
            nc.sync.dma_start(scl3_sb[:], scl3)

            with tc.tile_pool(name="selp", bufs=1) as selp:
                whh3_sb = selp.tile([128, 3 * KC * G], F8)
                nc.sync.dma_start(
                    whh3_sb[:].rearrange("p (q kc m) -> p q kc m", q=3, kc=KC),
                    whh3.rearrange("q (kc p) m -> p q kc m", p=128),
                )
                wih3_sb = selp.tile([128, 3 * XC2 * G], F8)
                nc.sync.dma_start(
                    wih3_sb[:].rearrange("p (q kx m) -> p q kx m", q=3, kx=XC2),
                    wih3.rearrange("q (kx p) m -> p q kx m", p=128),
                )
                wproj3_sb = selp.tile([128, 3 * EC * X2E], F16)
                nc.sync.dma_start(
                    wproj3_sb[:].rearrange("p (q kx m) -> p q kx m", q=3, kx=EC),
                    wproj3.rearrange("q (kx p) m -> p q kx m", p=128),
                )
                b2_3_sb = selp.tile([128, 3 * MC], F32)
                nc.sync.dma_start(
                    b2_3_sb[:].rearrange("p (q c o) -> p q c o", q=3, c=MC),
                    bias2_3.rearrange("q (c p) o -> p q c o", p=128),
                )
                bp3_sb = selp.tile([128, 3 * XC2], F32)
                nc.sync.dma_start(
                    bp3_sb[:].rearrange("p (q c o) -> p q c o", q=3, c=XC2),
                    bproj3.rearrange("q (c p) o -> p q c o", p=128),
                )

                def sel3(dst, src_sb, width, tmp_tag, dtype):
                    sv = lambda q: selv_sb[:, q : q + 1]
                    t1_ = selp.tile([128, width], dtype, tag=tmp_tag)
                    nc.vector.tensor_scalar_mul(
                        t1_[:], src_sb[:, 0:width], sv(0)
                    )
                    t2_ = selp.tile([128, width], dtype, tag=tmp_tag + "b")
                    nc.vector.scalar_tensor_tensor(
                        t2_[:], src_sb[:, width : 2 * width], sv(1), t1_[:],
                        MUL, ADD,
                    )
                    nc.vector.scalar_tensor_tensor(
                        dst, src_sb[:, 2 * width : 3 * width], sv(2), t2_[:],
                        MUL, ADD,
                    )

                sel3(whh8_sb[:], whh3_sb, KC * G, "twhh", F8)
                sel3(wih2_sb[:], wih3_sb, XC2 * G, "twih", F8)
                sel3(wproj_sb[:], wproj3_sb, EC * X2E, "twp", F16)
                sel3(bias2_sb[:], b2_3_sb, MC, "tb2", F32)
                sel3(bproj_sb[:], bp3_sb, XC2, "tbp", F32)
                sel3(sc_sb[:, 0:1], scl3_sb, 1, "ts1", F32)      # invs
                sel3(sc_sb[:, 1:2], scl3_sb[:, 3:], 1, "ts2", F32)  # xsc
                nc.vector.tensor_copy(sc_sb[:, 2:4], scl3_sb[:, 6:8])

            # head weights
            sw_sb = wts.tile([128, KC2 * H], F8)
            nc.sync.dma_start(
                sw_sb[:].rearrange("p (k m) -> p k m", k=KC2),
                sum_wT.rearrange("(k p) m -> p k m", p=128),
            )
            sb_sb = wts.tile([128, DC], F32)
            nc.sync.dma_start(
                sb_sb[:].rearrange("p (c o) -> p c o", o=1),
                sum_b.rearrange("(c p) o -> p c o", p=128),
            )
            ow_sb = wts.tile([128, DC * NA], F16)
            nc.sync.dma_start(
                ow_sb[:].rearrange("p (c a) -> p c a", c=DC),
                out_wT.rearrange("(c p) a -> p c a", p=128),
            )
            ob_sb = wts.tile([128, NA], F32)
            nc.sync.dma_start(ob_sb[:], out_bt)

            invs_ap = sc_sb[:, 0:1]
            xsc_ap = sc_sb[:, 1:2]
            esc_ap = sc_sb[:, 2:3]
            hsc_ap = sc_sb[:, 3:4]

            # precompute XCT = S*(WIH @ relu(Wproj @ ecatT + bproj) + bias2)
            for tc_i in range(T // TCH):
                tsl = slice(tc_i * TCH, (tc_i + 1) * TCH)
                x2_sb = x2p.tile([128, XC2 * TCH], F16)
                for mx in range(XC2):
                    ps = psp.tile([128, TCH], F32)
                    for kx in range(EC):
                        nc.tensor.matmul(
                            ps[:],
                            wproj_sb[
                                :, kx * X2E + mx * 128 : kx * X2E + (mx + 1) * 128
                            ],
                            ecat_sb[:, kx * T + tc_i * TCH : kx * T + (tc_i + 1) * TCH],
                            start=(kx == 0),
                            stop=(kx == EC - 1),
                        )
                    nc.scalar.activation(
                        x2_sb[:, mx * TCH : (mx + 1) * TCH],
                        ps[:],
                        AF.Relu,
                        bias=bproj_sb[:, mx : mx + 1],
                        scale=esc_ap,
                    )
                for m in range(MC):
                    ps = psp.tile([128, TCH], F32)
                    for kx in range(XC2):
                        nc.tensor.matmul(
                            ps[:],
                            wih2_sb[:, kx * G + m * 128 : kx * G + (m + 1) * 128],
                            x2_sb[:, kx * TCH : (kx + 1) * TCH],
                            start=(kx == 0),
                            stop=(kx == XC2 - 1),
                        )
                    xct_t = x2p.tile([128, TCH], F32, tag="xctout")
                    nc.scalar.activation(
                        xct_t[:], ps[:], AF.Identity, bias=bias2_sb[:, m : m + 1],
                        scale=xsc_ap,
                    )
                    nc.sync.dma_start(xct_d[m, :, tsl], xct_t[:])

            # sequential recurrence, software-pipelined XC prefetch
            h_cur = statep.tile([128, KC], F16)
            c_sb = statep.tile([128, KC], F32)
            tmp32 = statep.tile([128, KC], F32)
            nc.sync.dma_start(tmp32[:], h0[:])
            nc.vector.tensor_copy(h_cur[:], tmp32[:])
            nc.sync.dma_start(c_sb[:], c0[:])

            xcA = xcbp.tile([128, MC * U], F32, tag="xcA")
            xcB = xcbp.tile([128, MC * U], F32, tag="xcB")
            nc.sync.dma_start(
                xcA[:].rearrange("p (m u) -> p m u", m=MC),
                xct_d[:, :, 0:U].rearrange("m p u -> p m u"),
            )

            # persistent gate-psum banks (one per half); preloaded with the
            # 32 steps' x-contributions (gate-interleaved layout: col u*16+m),
            # PE accumulates on top with start=False. The preload for each
            # bank is emitted mid-way through the *other* half's step loop.
            gpsA = gpsp.tile([128, U * 16], F32, tag="psA")
            gpsB = gpsp.tile([128, U * 16], F32, tag="psB")

            def preload(psb, xc_sb):
                nc.vector.tensor_copy(
                    psb[:].rearrange("p (u m) -> p u m", m=16),
                    xc_sb[:].rearrange("p (m u) -> p u m", m=MC),
                )

            preload(gpsA, xcA)

            def half(xc_sb, psb, hist_ap, tag, pre_next):
                hist_t = histbp.tile([128, KC * U], F16, tag="h" + tag)
                hist_r = hist_t[:].rearrange("p (k u) -> p u k", k=KC)
                nc.vector.tensor_copy(hist_r[:, 0, :], h_cur[:])
                for u in range(U):
                    if u == 8:
                        pre_next()
                    base = u * 16
                    for grp in range(4):  # f, i, g, o
                        for m in range(grp * 4, grp * 4 + 4):
                            for kc in range(KC):
                                nc.tensor.matmul(
                                    psb[:, base + m : base + m + 1],
                                    whh8_sb[
                                        :, kc * G + m * 128 : kc * G + (m + 1) * 128
                                    ],
                                    hist_t[:, kc * U + u : kc * U + u + 1],
                                    start=False,
                                    stop=(kc == KC - 1),
                                )
                        if not pointwise:
                            if grp == 3:
                                so = ewp.tile([128, 4], F32, tag="so")
                                nc.scalar.activation(
                                    so[:], psb[:, base + 12 : base + 16],
                                    AF.Sigmoid, scale=invs_ap,
                                )
                            continue
                        if grp == 0:
                            sf = ewp.tile([128, 4], F32, tag="sf")
                            nc.scalar.activation(
                                sf[:], psb[:, base : base + 4], AF.Sigmoid,
                                scale=invs_ap,
                            )
                            t2 = ewp.tile([128, 4], F32, tag="t2")
                            nc.vector.tensor_mul(t2[:], sf[:], c_sb[:])
                        elif grp == 1:
                            si = ewp.tile([128, 4], F32, tag="si")
                            nc.scalar.activation(
                                si[:], psb[:, base + 4 : base + 8], AF.Sigmoid,
                                scale=invs_ap,
                            )
                        elif grp == 2:
                            tg = ewp.tile([128, 4], F32, tag="tg")
                            nc.scalar.activation(
                                tg[:], psb[:, base + 8 : base + 12], AF.Tanh,
                                scale=invs_ap,
                            )
                            t1 = ewp.tile([128, 4], F32, tag="t1")
                            nc.vector.tensor_mul(t1[:], si[:], tg[:])
                            nc.vector.tensor_add(c_sb[:], t1[:], t2[:])
                            tc2 = ewp.tile([128, 4], F32, tag="tc2")
                            nc.scalar.activation(tc2[:], c_sb[:], AF.Tanh)
                        else:
                            so = ewp.tile([128, 4], F32, tag="so")
                            nc.scalar.activation(
                                so[:], psb[:, base + 12 : base + 16], AF.Sigmoid,
                                scale=invs_ap,
                            )
                            if u < U - 1:
                                nc.vector.tensor_mul(
                                    hist_r[:, u + 1, :], so[:], tc2[:]
                                )
                            else:
                                nc.vector.tensor_mul(h_cur[:], so[:], tc2[:])
                # fp8 export of this block's history (x128, |h|<1 so safe)
                hist8 = histbp.tile([128, KC * U], F8E4, tag="h8" + tag)
                nc.scalar.activation(hist8[:], hist_t[:], AF.Identity, scale=SH)
                nc.sync.dma_start(
                    hist_ap.rearrange("k p u -> p k u"),
                    hist8[:].rearrange("p (k u) -> p k u", k=KC),
                )

            with tc.For_i(0, t_loop, 2 * U, hint_engines=(mybir.EngineType.PE,)) as iv:
                nc.sync.dma_start(
                    xcB[:].rearrange("p (m u) -> p m u", m=MC),
                    xct_d[:, :, U:][:, :, bass.ds(iv, U)].rearrange("m p u -> p m u"),
                )
                half(
                    xcA, gpsA, hist_d[:, :, bass.ds(iv, U)], "A",
                    lambda: preload(gpsB, xcB),
                )
                nc.sync.dma_start(
                    xcA[:].rearrange("p (m u) -> p m u", m=MC),
                    xct_d[:, :, 2 * U :][:, :, bass.ds(iv, U)].rearrange(
                        "m p u -> p m u"
                    ),
                )
                half(
                    xcB, gpsB, hist_d[:, :, U:][:, :, bass.ds(iv, U)], "B",
                    lambda: preload(gpsA, xcA),
                )

            # re-layout history into AllToAll shard-major order and exchange:
            # after A2A, out[r] on core c holds rank r's history columns for
            # time shard c. Ranks 0-2 are chains stk/buf/hist.
            a2a_in = dramp.tile([NCORES, KC, 128, TS], F8E4)
            a2a_out = dramp.tile([NCORES, KC, 128, TS], F8E4)
            for s in range(NCORES):
                nc.sync.dma_start(a2a_in[s], hist_d[:, :, s * TS : (s + 1) * TS])
            nc.gpsimd.collective_compute(
                "AllToAll",
                mybir.AluOpType.bypass,
                replica_groups=[list(range(NCORES))],
                ins=[a2a_in.opt()],
                outs=[a2a_out.opt()],
            )

            # softmax head on this core's T/8 shard
            top_sb = wts.tile([128, KC2 * TS], F8E4)
            top_r = top_sb[:].rearrange("p (k t) -> p k t", k=KC2)
            for q in range(3):
                nc.sync.dma_start(
                    top_r[:, q * KC : (q + 1) * KC, :],
                    a2a_out[q].rearrange("k p t -> p k t"),
                )
            st_sb = wts.tile([128, DC * TS], F16)
            for dc in range(DC):
                ps = psp.tile([128, TS], F32)
                for kc in range(KC2):
                    nc.tensor.matmul(
                        ps[:],
                        sw_sb[:, kc * H + dc * 128 : kc * H + (dc + 1) * 128],
                        top_sb[:, kc * TS : (kc + 1) * TS],
                        start=(kc == 0),
                        stop=(kc == KC2 - 1),
                    )
                nc.scalar.activation(
                    st_sb[:, dc * TS : (dc + 1) * TS],
                    ps[:],
                    AF.Tanh,
                    bias=sb_sb[:, dc : dc + 1],
                    scale=hsc_ap,
                )
            for tcc in range(TC):
                ps2 = p2p.tile([128, NA], F32)
                for dc in range(DC):
                    nc.tensor.matmul(
                        ps2[:],
                        st_sb[:, dc * TS + tcc * 128 : dc * TS + tcc * 128 + 128],
                        ow_sb[:, dc * NA : (dc + 1) * NA],
                        start=(dc == 0),
                        stop=(dc == DC - 1),
                    )
                L = ewp.tile([128, NA], F32, tag="hL")
                nc.vector.tensor_add(L[:], ps2[:], ob_sb[:])
                mx = ewp.tile([128, 1], F32, tag="hmx")
                nc.vector.reduce_max(mx[:], L[:], axis=mybir.AxisListType.X)
                D = ewp.tile([128, NA], F32, tag="hD")
                nc.vector.tensor_scalar(
                    D[:], L[:], mx[:], None, mybir.AluOpType.subtract
                )
                Ex = ewp.tile([128, NA], F32, tag="hE")
                nc.scalar.activation(Ex[:], D[:], AF.Exp)
                s = ewp.tile([128, 1], F32, tag="hs")
                nc.vector.reduce_sum(s[:], Ex[:], axis=mybir.AxisListType.X)
                ls = ewp.tile([128, 1], F32, tag="hls")
                nc.scalar.activation(ls[:], s[:], AF.Ln)
                O = ewp.tile([128, NA], F32, tag="hO")
                nc.vector.tensor_scalar(
                    O[:], D[:], ls[:], None, mybir.AluOpType.subtract
                )
                nc.sync.dma_start(outd[tcc * 128 : (tcc + 1) * 128, :], O[:])

    _split_excess_waits(nc)
    return nc


def _make_runner(nc, n_cores=NCORES, repl_names=frozenset()):
    import jax
    from jax.sharding import Mesh, PartitionSpec
    from jax.experimental.shard_map import shard_map
    from concourse import bass2jax
    from concourse.bass2jax import _bass_exec_p, partition_id_tensor

    bass2jax.install_neuronx_cc_hook()

    partition_name = nc.partition_id_tensor.name if nc.partition_id_tensor else None
    in_names, out_names, out_avals, zero_outs = [], [], [], []
    for alloc in nc.m.functions[0].allocations:
        if not isinstance(alloc, mybir.MemoryLocationSet):
            continue
        name = alloc.memorylocations[0].name
        if alloc.kind == "ExternalInput":
            if name != partition_name:
                in_names.append(name)
        elif alloc.kind == "ExternalOutput":
            shape = tuple(alloc.tensor_shape)
            dtype = mybir.dt.np(alloc.dtype)
            out_names.append(name)
            out_avals.append(jax.core.ShapedArray(shape, dtype))
            zero_outs.append(np.zeros(shape, dtype))
    n_params = len(in_names)
    all_in = list(in_names) + list(out_names) + (
        [partition_name] if partition_name else []
    )

    def _body(*args):
        operands = list(args)
        if partition_name:
            operands.append(partition_id_tensor())
        return tuple(
            _bass_exec_p.bind(
                *operands,
                out_avals=tuple(out_avals),
                in_names=tuple(all_in),
                out_names=tuple(out_names),
                lowering_input_output_aliases=(),
                sim_require_finite=True,
                sim_require_nnan=True,
                nc=nc,
            )
        )

    devices = jax.devices()[:n_cores]
    mesh = Mesh(np.asarray(devices), ("core",))
    in_specs = tuple(
        PartitionSpec() if n in repl_names else PartitionSpec("core")
        for n in in_names
    ) + (PartitionSpec("core"),) * len(out_names)
    fn = jax.jit(
        shard_map(
            _body,
            mesh=mesh,
            in_specs=in_specs,
            out_specs=(PartitionSpec("core"),) * len(out_names),
            check_rep=False,
        ),
        keep_unused=True,
    )

    def run(in_maps):
        import jax

        concat_in = []
        for n in in_names:
            if n in repl_names:
                concat_in.append(np.asarray(in_maps[0][n]))
            else:
                concat_in.append(
                    np.concatenate(
                        [np.asarray(in_maps[c][n]) for c in range(n_cores)], axis=0
                    )
                )
        concat_zeros = [
            np.zeros((n_cores * z.shape[0], *z.shape[1:]), z.dtype)
            for z in zero_outs
        ]
        out = fn(*(concat_in + concat_zeros))
        jax.block_until_ready(out)
        return [
            {
                name: np.asarray(out[i]).reshape(n_cores, *out_avals[i].shape)[c]
                for i, name in enumerate(out_names)
            }
            for c in range(n_cores)
        ]

    run.fn = fn
    run.spec = (in_names, out_names, out_avals, zero_outs, n_cores, repl_names)
    return run


_CACHE = {}


def _runner():
    if "f" not in _CACHE:
        _CACHE["f"] = _make_runner(_build_fused(), repl_names=REPL)
    return _CACHE["f"]


def _q8(x, target=14.0, clip=15.5):
    """Quantize to fp8-e3m4 with a power-of-two scale; returns (q, S)."""
    m = max(float(np.abs(x).max()), 1e-6)
    S = float(2.0 ** np.floor(np.log2(target / m)))
    return np.clip(x * S, -clip, clip).astype(E3M4), S


# gate-order permutation (i,f,g,o) -> (f,i,g,o), applied to weight rows
_PERM = np.concatenate(
    [np.arange(512, 1024), np.arange(0, 512), np.arange(1024, 1536),
     np.arange(1536, 2048)]
)


def _prep_inputs(inputs):
    words = np.asarray(inputs["words"]).astype(np.int64)
    pos_tags = np.asarray(inputs["pos_tags"]).astype(np.int64)
    actions = np.asarray(inputs["actions"]).astype(np.int64)

    # host-side embedding gather (4096 of 100k rows), zero-padded to 512
    ecat = np.zeros((T, E), np.float32)
    ecat[:, 0:300] = np.asarray(inputs["word_emb"])[words]
    ecat[:, 300:332] = np.asarray(inputs["pos_emb"])[pos_tags]
    ecat[:, 332:396] = np.asarray(inputs["act_emb"])[actions]
    ecat8, S_e = _q8(ecat.T)

    sw8, S_sw = _q8(np.asarray(inputs["sum_w"], np.float32).T)

    whh3 = np.empty((3, H, G), E3M4)
    wih3 = np.empty((3, X2E, G), E3M4)
    wproj3 = np.zeros((3, E, X2E), np.float16)
    bias2_3 = np.empty((3, G), np.float32)
    bproj3 = np.zeros((3, X2E), np.float32)
    scl3 = np.zeros((128, 8), np.float32)
    h0s, c0s = [], []
    for q, (pre, kind) in enumerate([("stk", "w"), ("buf", "w"), ("hist", "a")]):
        wih = np.asarray(inputs[f"{pre}_wih"], np.float32)[_PERM]
        whh = np.asarray(inputs[f"{pre}_whh"], np.float32)[_PERM]
        bias = (
            np.asarray(inputs[f"{pre}_bih"], np.float32)
            + np.asarray(inputs[f"{pre}_bhh"], np.float32)
        )[_PERM]
        whh8, S = _q8(whh)
        wih8, S_w2 = _q8(wih)
        whh3[q] = whh8.T
        wih3[q] = wih8.T
        bias2_3[q] = bias * S
        if kind == "w":
            wproj3[q, 0:332, :] = np.asarray(inputs["w2e_w"]).T
            bproj3[q] = np.asarray(inputs["w2e_b"], np.float32)
        else:
            wproj3[q, 332:396, :] = np.asarray(inputs["a2e_w"]).T
            bproj3[q] = np.asarray(inputs["a2e_b"], np.float32)
        scl3[:, q] = 1.0 / S
        scl3[:, 3 + q] = S / S_w2
        h0s.append(
            np.ascontiguousarray(
                np.asarray(inputs[f"{pre}_h0"]).reshape(4, 128).T
            ).astype(np.float32)
        )
        c0s.append(
            np.ascontiguousarray(
                np.asarray(inputs[f"{pre}_c0"]).reshape(4, 128).T
            ).astype(np.float32)
        )
    scl3[:, 6] = 1.0 / S_e
    scl3[:, 7] = 1.0 / (S_sw * SH)

    shared_vals = dict(
        ecatT=np.ascontiguousarray(ecat8),
        sum_wT=np.ascontiguousarray(sw8),
        out_wT=np.ascontiguousarray(np.asarray(inputs["out_w"]).T).astype(
            np.float16
        ),
        out_bt=np.broadcast_to(np.asarray(inputs["out_b"]), (128, NA))
        .astype(np.float32)
        .copy(),
        sum_b=np.asarray(inputs["sum_b"]).reshape(H, 1).astype(np.float32),
        whh3=whh3,
        wih3=wih3,
        wproj3=wproj3,
        bias2_3=bias2_3.reshape(3, G, 1),
        bproj3=bproj3.reshape(3, X2E, 1),
        scl3=scl3,
    )
    in_maps = []
    for c in range(NCORES):
        q = c % 3
        sv = np.zeros((128, 3), np.float32)
        sv[:, q] = 1.0
        in_maps.append(dict(selv=sv, h0=h0s[q], c0=c0s[q], **shared_vals))
    return in_maps


def kernel(**inputs):
    run = _runner()
    in_maps = _prep_inputs(inputs)
    res = run(in_maps)
    return np.concatenate([res[c]["logp"] for c in range(NCORES)], axis=0).astype(
        np.float32
    )
